# revision 6
# baseline (speedup 1.0000x reference)
"""VQ codebook quantizer (AtomQuantizer) on 8 TRN2 NeuronCores.

Data-parallel over the token dim: each core scores its 8192-row shard of x
against the full 4096x512 codebook on the TensorEngine, finds the argmin +
min distance per row with the DVE top-8 instructions, and gathers the
selected codebook rows with an indirect DMA. The scalar loss is assembled
on the host from the per-row min scores (an 8-way scalar all-reduce).

Matmul precision: fp32 matmuls stream at 4 cycles/column on TRN2, so the
cross term 2*x@e^T is computed as three 1-cycle/column fp16 matmuls using
an error-compensated hi/lo split (x = xh + xl, e = eh + el):
    x*e ~= xh*eh + xl*eh + xh*el        (xl*el ~ 2^-22 relative, dropped)
A 256x prescale keeps the lo parts clear of the fp16 subnormal range, so
the result is fp32-grade and the argmin matches the fp32 reference exactly.
"""

import os
import sys

sys.path.insert(0, "/opt/trn_rl_repo")

import numpy as np

N, D, K = 65536, 512, 4096
NCORES = 8
ROWS = N // NCORES  # 8192 rows per core
RT = ROWS // 128  # 64 row-tiles per core
PH_ROWS = 1024  # rows per x double-buffer phase
PHASES = ROWS // PH_ROWS
RT_PH = PH_ROWS // 128
NTILE = 512  # moving-operand columns per matmul (one PSUM bank)
CCH = K // NTILE  # 8 code chunks
DCH = D // 128  # 4 contraction chunks
SCALE = np.float32(256.0)
COMMITMENT_COST = 0.25

VARIANT = os.environ.get("VQ_VARIANT", "fp16x2")  # "fp16x2" | "fp32"

_CACHE = {}


def _build(variant, iters=1):
    import contextlib

    import concourse.bass as bass
    import concourse.mybir as mybir
    import concourse.tile as tile
    from concourse import bacc

    f16 = mybir.dt.float16
    f32 = mybir.dt.float32
    u32 = mybir.dt.uint32
    xdt = f16 if variant == "fp16x2" else f32

    nc = bacc.Bacc(None, target_bir_lowering=False, debug=False)

    xh_e = nc.dram_tensor("xh", [D, ROWS], xdt, kind="ExternalInput")
    eh_e = nc.dram_tensor("eh", [D, K], xdt, kind="ExternalInput")
    if variant == "fp16x2":
        xl_e = nc.dram_tensor("xl", [D, ROWS], xdt, kind="ExternalInput")
        el_e = nc.dram_tensor("el", [D, K], xdt, kind="ExternalInput")
    nn_e = nc.dram_tensor("negnorm", [128, K], f32, kind="ExternalInput")
    emb_e = nc.dram_tensor("emb", [K, D], f32, kind="ExternalInput")
    q_e = nc.dram_tensor("q", [ROWS, D], f32, kind="ExternalOutput")
    mv_e = nc.dram_tensor("mv", [128, RT], f32, kind="ExternalOutput")

    with tile.TileContext(nc) as tc:
        with (
            tc.tile_pool(name="const", bufs=1) as constp,
            tc.tile_pool(name="xpool", bufs=2) as xpool,
            tc.tile_pool(name="scores", bufs=2) as spool,
            tc.tile_pool(name="small", bufs=3) as smallp,
            tc.tile_pool(name="qpool", bufs=3) as qpool,
            tc.tile_pool(name="psum", bufs=4, space="PSUM") as psum,
        ):
            eh = constp.tile([128, DCH, K], xdt, tag="eh")
            nc.sync.dma_start(eh[:], eh_e[:].rearrange("(d p) n -> p d n", p=128))
            if variant == "fp16x2":
                el = constp.tile([128, DCH, K], xdt, tag="el")
                nc.sync.dma_start(el[:], el_e[:].rearrange("(d p) n -> p d n", p=128))
            nn = constp.tile([128, K], f32, tag="nn")
            nc.sync.dma_start(nn[:], nn_e[:])
            mv = constp.tile([128, RT], f32, tag="mv")

            # iters > 1 repeats the whole compute body for loop-differencing
            # timing; the same instructions run N times over the same data.
            loop_ctx = (
                tc.For_i(0, iters, 1) if iters > 1 else contextlib.nullcontext()
            )
            with loop_ctx:
                _body(nc, bass, mybir, variant, xpool, spool, smallp, qpool,
                      psum, xdt, f32, u32, eh, el if variant == "fp16x2" else None,
                      nn, mv, xh_e, xl_e if variant == "fp16x2" else None,
                      emb_e, q_e)
            nc.sync.dma_start(mv_e[:], mv[:])

    nc.compile()
    return nc


def _body(nc, bass, mybir, variant, xpool, spool, smallp, qpool, psum, xdt,
          f32, u32, eh, el, nn, mv, xh_e, xl_e, emb_e, q_e):
    if True:
            for ph in range(PHASES):
                lo, hi = ph * PH_ROWS, (ph + 1) * PH_ROWS
                xh_t = xpool.tile([128, DCH, PH_ROWS], xdt, tag="xh")
                nc.sync.dma_start(
                    xh_t[:], xh_e[:, lo:hi].rearrange("(d p) n -> p d n", p=128)
                )
                if variant == "fp16x2":
                    xl_t = xpool.tile([128, DCH, PH_ROWS], xdt, tag="xl")
                    nc.sync.dma_start(
                        xl_t[:], xl_e[:, lo:hi].rearrange("(d p) n -> p d n", p=128)
                    )
                    passes = [(xh_t, eh), (xl_t, eh), (xh_t, el)]
                else:
                    passes = [(xh_t, eh)]

                for r in range(RT_PH):
                    rg = ph * RT_PH + r
                    sc = spool.tile([128, K], f32, tag="sc")
                    for c in range(CCH):
                        ps = psum.tile([128, NTILE], f32, tag="ps")
                        n_mm = len(passes) * DCH
                        i_mm = 0
                        for stat, mov in passes:
                            for d in range(DCH):
                                nc.tensor.matmul(
                                    ps[:],
                                    stat[:, d, r * 128 : (r + 1) * 128],
                                    mov[:, d, c * NTILE : (c + 1) * NTILE],
                                    start=(i_mm == 0),
                                    stop=(i_mm == n_mm - 1),
                                )
                                i_mm += 1
                        nc.vector.tensor_tensor(
                            out=sc[:, c * NTILE : (c + 1) * NTILE],
                            in0=ps[:],
                            in1=nn[:, c * NTILE : (c + 1) * NTILE],
                            op=mybir.AluOpType.add,
                        )
                    m8 = smallp.tile([128, 8], f32, tag="m8")
                    i8 = smallp.tile([128, 8], u32, tag="i8")
                    nc.vector.max(m8[:], sc[:])
                    nc.vector.max_index(i8[:], m8[:], sc[:])
                    nc.vector.tensor_copy(mv[:, rg : rg + 1], m8[:, 0:1])
                    qt = qpool.tile([128, D], f32, tag="qt")
                    nc.gpsimd.indirect_dma_start(
                        out=qt[:],
                        out_offset=None,
                        in_=emb_e[:],
                        in_offset=bass.IndirectOffsetOnAxis(ap=i8[:, 0:1], axis=0),
                    )
                    nc.scalar.dma_start(q_e[rg * 128 : (rg + 1) * 128, :], qt[:])


def _split16(a):
    ah = a.astype(np.float16)
    al = (a - ah.astype(np.float32)).astype(np.float16)
    return ah, al


def kernel(x, embeddings, trace=False):
    from concourse.bass_utils import run_bass_kernel_spmd

    x = np.ascontiguousarray(np.asarray(x, dtype=np.float32))
    e = np.ascontiguousarray(np.asarray(embeddings, dtype=np.float32))

    variant = VARIANT
    if variant not in _CACHE:
        _CACHE[variant] = _build(variant)
    nc = _CACHE[variant]

    es = e * SCALE
    negnorm = -np.sum(es.astype(np.float64) ** 2, axis=1)
    negnorm_b = np.ascontiguousarray(
        np.broadcast_to(negnorm.astype(np.float32), (128, K))
    )
    if variant == "fp16x2":
        eh, el = _split16(es)
        ehT = np.ascontiguousarray(eh.T)
        elT = np.ascontiguousarray(el.T)
    else:
        ehT = np.ascontiguousarray(es.T)

    in_maps = []
    for i in range(NCORES):
        # 2*SCALE: the cross term needs the factor 2 from -2*x.e; folding it
        # into the x-side prescale makes the device compute
        # S^2 * (2*x.e - ||e||^2) with no extra instructions.
        xs = x[i * ROWS : (i + 1) * ROWS] * (2.0 * SCALE)
        m = {"negnorm": negnorm_b, "emb": e, "eh": ehT}
        if variant == "fp16x2":
            xh, xl = _split16(xs)
            m["xh"] = np.ascontiguousarray(xh.T)
            m["xl"] = np.ascontiguousarray(xl.T)
            m["el"] = elT
        else:
            m["xh"] = np.ascontiguousarray(xs.T)
        in_maps.append(m)

    res = run_bass_kernel_spmd(nc, in_maps, list(range(NCORES)), trace=trace)

    q = np.concatenate([res.results[i]["q"] for i in range(NCORES)], axis=0)

    # loss = 1.25 * mean((q - x)^2); sum_i ||q_i - x_i||^2 = ||x||_F^2 + sum_i min_score_i
    # where min_score_i = min_k(||e_k||^2 - 2 x_i . e_k) = -mv / SCALE^2.
    sum_x2 = 0.0
    for i in range(0, N, 8192):
        xb = x[i : i + 8192].astype(np.float64)
        sum_x2 += float(np.einsum("ij,ij->", xb, xb))
    sum_min = 0.0
    for i in range(NCORES):
        sum_min += -np.sum(res.results[i]["mv"].astype(np.float64)) / float(
            SCALE
        ) ** 2
    loss = np.float32((1.0 + COMMITMENT_COST) * (sum_x2 + sum_min) / (N * D))

    if trace:
        kernel.last_exec_time_ns = res.exec_time_ns
        kernel.last_results = res
    return (q, loss)


# revision 10
# speedup vs baseline: 19.5477x; 19.5477x over previous
"""VQ codebook quantizer (AtomQuantizer) on 8 TRN2 NeuronCores.

Data-parallel over the token dim: each core scores its 8192-row shard of x
against the full 4096x512 codebook on the TensorEngine, finds the argmin +
min distance per row with the DVE top-8 instructions, and gathers the
selected codebook rows with an indirect DMA. The scalar loss is assembled
on the host from the per-row min scores (an 8-way scalar all-reduce).

Matmul precision: fp32 matmuls stream at 4 cycles/column on TRN2, so the
cross term 2*x@e^T is computed as three 1-cycle/column fp16 matmuls using
an error-compensated hi/lo split (x = xh + xl, e = eh + el):
    x*e ~= xh*eh + xl*eh + xh*el        (xl*el ~ 2^-22 relative, dropped)
A 256x prescale keeps the lo parts clear of the fp16 subnormal range, so
the result is fp32-grade and the argmin matches the fp32 reference exactly.
"""

import os
import sys

sys.path.insert(0, "/opt/trn_rl_repo")

import numpy as np

N, D, K = 65536, 512, 4096
NCORES = 8
ROWS = N // NCORES  # 8192 rows per core
RT = ROWS // 128  # 64 row-tiles per core
PH_ROWS = 1024  # rows per x double-buffer phase
PHASES = ROWS // PH_ROWS
RT_PH = PH_ROWS // 128
NTILE = 512  # moving-operand columns per matmul (one PSUM bank)
CCH = K // NTILE  # 8 code chunks
DCH = D // 128  # 4 contraction chunks
SCALE = np.float32(256.0)
COMMITMENT_COST = 0.25

VARIANT = os.environ.get("VQ_VARIANT", "fp16x2")  # "fp16x2" | "fp32"

_CACHE = {}


def _build(variant, iters=1, rows=ROWS):
    import contextlib

    rt = rows // 128
    phases = rows // PH_ROWS

    import concourse.bass as bass
    import concourse.mybir as mybir
    import concourse.tile as tile
    from concourse import bacc

    f16 = mybir.dt.float16
    f32 = mybir.dt.float32
    u32 = mybir.dt.uint32
    xdt = f16 if variant == "fp16x2" else f32

    nc = bacc.Bacc(None, target_bir_lowering=False, debug=False)

    xh_e = nc.dram_tensor("xh", [D, rows], xdt, kind="ExternalInput")
    eh_e = nc.dram_tensor("eh", [D, K], xdt, kind="ExternalInput")
    if variant == "fp16x2":
        xl_e = nc.dram_tensor("xl", [D, rows], xdt, kind="ExternalInput")
        el_e = nc.dram_tensor("el", [D, K], xdt, kind="ExternalInput")
    nn_e = nc.dram_tensor("negnorm", [128, K], f32, kind="ExternalInput")
    emb_e = nc.dram_tensor("emb", [K, D], f32, kind="ExternalInput")
    q_e = nc.dram_tensor("q", [rows, D], f32, kind="ExternalOutput")
    mv_e = nc.dram_tensor("mv", [128, rt], f32, kind="ExternalOutput")

    with tile.TileContext(nc) as tc:
        with (
            tc.tile_pool(name="const", bufs=1) as constp,
            tc.tile_pool(name="xpool", bufs=2) as xpool,
            tc.tile_pool(name="scores", bufs=2) as spool,
            tc.tile_pool(name="small", bufs=3) as smallp,
            tc.tile_pool(name="qpool", bufs=3) as qpool,
            tc.tile_pool(name="psum", bufs=4, space="PSUM") as psum,
        ):
            # chunked code-table loads: the first matmuls only need code
            # chunk 0, so split the eh/el DMAs per 512-code chunk to shrink
            # the startup stall.
            eh = constp.tile([128, DCH, K], xdt, tag="eh")
            eh_r = eh_e[:].rearrange("(d p) n -> p d n", p=128)
            for c in range(CCH):
                cs = slice(c * NTILE, (c + 1) * NTILE)
                nc.scalar.dma_start(eh[:, :, cs], eh_r[:, :, cs])
            if variant == "fp16x2":
                el = constp.tile([128, DCH, K], xdt, tag="el")
                el_r = el_e[:].rearrange("(d p) n -> p d n", p=128)
                for c in range(CCH):
                    cs = slice(c * NTILE, (c + 1) * NTILE)
                    nc.scalar.dma_start(el[:, :, cs], el_r[:, :, cs])
            nn = constp.tile([128, K], f32, tag="nn")
            nc.scalar.dma_start(nn[:], nn_e[:])
            mv = constp.tile([128, rt], f32, tag="mv")

            # iters > 1 repeats the whole compute body for loop-differencing
            # timing; the same instructions run N times over the same data.
            loop_ctx = (
                tc.For_i(0, iters, 1) if iters > 1 else contextlib.nullcontext()
            )
            with loop_ctx:
                _body(nc, bass, mybir, variant, xpool, spool, smallp, qpool,
                      psum, xdt, f32, u32, eh, el if variant == "fp16x2" else None,
                      nn, mv, xh_e, xl_e if variant == "fp16x2" else None,
                      emb_e, q_e, phases)
            nc.sync.dma_start(mv_e[:], mv[:])

    nc.compile()
    return nc


def _body(nc, bass, mybir, variant, xpool, spool, smallp, qpool, psum, xdt,
          f32, u32, eh, el, nn, mv, xh_e, xl_e, emb_e, q_e, phases=PHASES):
    if True:
            for ph in range(phases):
                lo, hi = ph * PH_ROWS, (ph + 1) * PH_ROWS
                xh_t = xpool.tile([128, DCH, PH_ROWS], xdt, tag="xh")
                nc.sync.dma_start(
                    xh_t[:], xh_e[:, lo:hi].rearrange("(d p) n -> p d n", p=128)
                )
                if variant == "fp16x2":
                    xl_t = xpool.tile([128, DCH, PH_ROWS], xdt, tag="xl")
                    nc.sync.dma_start(
                        xl_t[:], xl_e[:, lo:hi].rearrange("(d p) n -> p d n", p=128)
                    )
                    passes = [(xh_t, eh), (xl_t, eh), (xh_t, el)]
                else:
                    passes = [(xh_t, eh)]

                for r in range(RT_PH):
                    rg = ph * RT_PH + r
                    sc = spool.tile([128, K], f32, tag="sc")
                    for c in range(CCH):
                        ps = psum.tile([128, NTILE], f32, tag="ps")
                        n_mm = len(passes) * DCH
                        i_mm = 0
                        for stat, mov in passes:
                            for d in range(DCH):
                                nc.tensor.matmul(
                                    ps[:],
                                    stat[:, d, r * 128 : (r + 1) * 128],
                                    mov[:, d, c * NTILE : (c + 1) * NTILE],
                                    start=(i_mm == 0),
                                    stop=(i_mm == n_mm - 1),
                                )
                                i_mm += 1
                        nc.vector.tensor_tensor(
                            out=sc[:, c * NTILE : (c + 1) * NTILE],
                            in0=ps[:],
                            in1=nn[:, c * NTILE : (c + 1) * NTILE],
                            op=mybir.AluOpType.add,
                        )
                    m8 = smallp.tile([128, 8], f32, tag="m8")
                    i8 = smallp.tile([128, 8], u32, tag="i8")
                    nc.vector.max(m8[:], sc[:])
                    nc.vector.max_index(i8[:], m8[:], sc[:])
                    nc.vector.tensor_copy(mv[:, rg : rg + 1], m8[:, 0:1])
                    qt = qpool.tile([128, D], f32, tag="qt")
                    nc.gpsimd.indirect_dma_start(
                        out=qt[:],
                        out_offset=None,
                        in_=emb_e[:],
                        in_offset=bass.IndirectOffsetOnAxis(ap=i8[:, 0:1], axis=0),
                    )
                    nc.scalar.dma_start(q_e[rg * 128 : (rg + 1) * 128, :], qt[:])


def _split16(a):
    ah = a.astype(np.float16)
    al = (a - ah.astype(np.float32)).astype(np.float16)
    return ah, al


def kernel(x, embeddings, trace=False):
    from concourse.bass_utils import run_bass_kernel_spmd

    x = np.ascontiguousarray(np.asarray(x, dtype=np.float32))
    e = np.ascontiguousarray(np.asarray(embeddings, dtype=np.float32))

    variant = VARIANT
    if variant not in _CACHE:
        _CACHE[variant] = _build(variant)
    nc = _CACHE[variant]

    es = e * SCALE
    negnorm = -np.sum(es.astype(np.float64) ** 2, axis=1)
    negnorm_b = np.ascontiguousarray(
        np.broadcast_to(negnorm.astype(np.float32), (128, K))
    )
    if variant == "fp16x2":
        eh, el = _split16(es)
        ehT = np.ascontiguousarray(eh.T)
        elT = np.ascontiguousarray(el.T)
    else:
        ehT = np.ascontiguousarray(es.T)

    in_maps = []
    for i in range(NCORES):
        # 2*SCALE: the cross term needs the factor 2 from -2*x.e; folding it
        # into the x-side prescale makes the device compute
        # S^2 * (2*x.e - ||e||^2) with no extra instructions.
        xs = x[i * ROWS : (i + 1) * ROWS] * (2.0 * SCALE)
        m = {"negnorm": negnorm_b, "emb": e, "eh": ehT}
        if variant == "fp16x2":
            xh, xl = _split16(xs)
            m["xh"] = np.ascontiguousarray(xh.T)
            m["xl"] = np.ascontiguousarray(xl.T)
            m["el"] = elT
        else:
            m["xh"] = np.ascontiguousarray(xs.T)
        in_maps.append(m)

    res = run_bass_kernel_spmd(nc, in_maps, list(range(NCORES)), trace=trace)

    q = np.concatenate([res.results[i]["q"] for i in range(NCORES)], axis=0)

    # loss = 1.25 * mean((q - x)^2); sum_i ||q_i - x_i||^2 = ||x||_F^2 + sum_i min_score_i
    # where min_score_i = min_k(||e_k||^2 - 2 x_i . e_k) = -mv / SCALE^2.
    sum_x2 = 0.0
    for i in range(0, N, 8192):
        xb = x[i : i + 8192].astype(np.float64)
        sum_x2 += float(np.einsum("ij,ij->", xb, xb))
    sum_min = 0.0
    for i in range(NCORES):
        sum_min += -np.sum(res.results[i]["mv"].astype(np.float64)) / float(
            SCALE
        ) ** 2
    loss = np.float32((1.0 + COMMITMENT_COST) * (sum_x2 + sum_min) / (N * D))

    if trace:
        kernel.last_exec_time_ns = res.exec_time_ns
        kernel.last_results = res
    return (q, loss)


# revision 12
# speedup vs baseline: 19.8065x; 1.0132x over previous
"""VQ codebook quantizer (AtomQuantizer) on 8 TRN2 NeuronCores.

Data-parallel over the token dim: each core scores its 8192-row shard of x
against the full 4096x512 codebook on the TensorEngine, finds the argmin +
min distance per row with the DVE top-8 instructions, and gathers the
selected codebook rows with an indirect DMA. The scalar loss is assembled
on the host from the per-row min scores (an 8-way scalar all-reduce).

Matmul precision: fp32 matmuls stream at 4 cycles/column on TRN2, so the
cross term 2*x@e^T is computed as three 1-cycle/column fp16 matmuls using
an error-compensated hi/lo split (x = xh + xl, e = eh + el):
    x*e ~= xh*eh + xl*eh + xh*el        (xl*el ~ 2^-22 relative, dropped)
A 256x prescale keeps the lo parts clear of the fp16 subnormal range, so
the result is fp32-grade and the argmin matches the fp32 reference exactly.
"""

import os
import sys

sys.path.insert(0, "/opt/trn_rl_repo")

import numpy as np

N, D, K = 65536, 512, 4096
NCORES = 8
ROWS = N // NCORES  # 8192 rows per core
RT = ROWS // 128  # 64 row-tiles per core
PH_ROWS = 1024  # rows per x double-buffer phase
PHASES = ROWS // PH_ROWS
RT_PH = PH_ROWS // 128
NTILE = 512  # moving-operand columns per matmul (one PSUM bank)
CCH = K // NTILE  # 8 code chunks
DCH = D // 128  # 4 contraction chunks
SCALE = np.float32(256.0)
COMMITMENT_COST = 0.25

VARIANT = os.environ.get("VQ_VARIANT", "fp16x2")  # "fp16x2" | "fp32"

_CACHE = {}


def _build(variant, iters=1, rows=ROWS):
    import contextlib

    rt = rows // 128
    phases = rows // PH_ROWS

    import concourse.bass as bass
    import concourse.mybir as mybir
    import concourse.tile as tile
    from concourse import bacc

    f16 = mybir.dt.float16
    f32 = mybir.dt.float32
    u32 = mybir.dt.uint32
    xdt = f16 if variant == "fp16x2" else f32

    nc = bacc.Bacc(None, target_bir_lowering=False, debug=False)

    xh_e = nc.dram_tensor("xh", [D, rows], xdt, kind="ExternalInput")
    eh_e = nc.dram_tensor("eh", [D, K], xdt, kind="ExternalInput")
    if variant == "fp16x2":
        xl_e = nc.dram_tensor("xl", [D, rows], xdt, kind="ExternalInput")
        el_e = nc.dram_tensor("el", [D, K], xdt, kind="ExternalInput")
    nn_e = nc.dram_tensor("negnorm", [128, K], f32, kind="ExternalInput")
    emb_e = nc.dram_tensor("emb", [K, D], f32, kind="ExternalInput")
    q_e = nc.dram_tensor("q", [rows, D], f32, kind="ExternalOutput")
    mv_e = nc.dram_tensor("mv", [128, rt], f32, kind="ExternalOutput")

    with tile.TileContext(nc) as tc:
        with (
            tc.tile_pool(name="const", bufs=1) as constp,
            tc.tile_pool(name="xpool", bufs=2) as xpool,
            tc.tile_pool(name="scores", bufs=int(os.environ.get("VQ_SC_BUFS", "2")) ) as spool,
            tc.tile_pool(name="small", bufs=3) as smallp,
            tc.tile_pool(name="qpool", bufs=3) as qpool,
            tc.tile_pool(name="psum", bufs=int(os.environ.get("VQ_PSUM_BUFS", "8")), space="PSUM") as psum,
        ):
            # chunked code-table loads: the first matmuls only need code
            # chunk 0, so split the eh/el DMAs per 512-code chunk to shrink
            # the startup stall.
            eh = constp.tile([128, DCH, K], xdt, tag="eh")
            eh_r = eh_e[:].rearrange("(d p) n -> p d n", p=128)
            for c in range(CCH):
                cs = slice(c * NTILE, (c + 1) * NTILE)
                nc.scalar.dma_start(eh[:, :, cs], eh_r[:, :, cs])
            if variant == "fp16x2":
                el = constp.tile([128, DCH, K], xdt, tag="el")
                el_r = el_e[:].rearrange("(d p) n -> p d n", p=128)
                for c in range(CCH):
                    cs = slice(c * NTILE, (c + 1) * NTILE)
                    nc.scalar.dma_start(el[:, :, cs], el_r[:, :, cs])
            nn = constp.tile([128, K], f32, tag="nn")
            nc.scalar.dma_start(nn[:], nn_e[:])
            mv = constp.tile([128, rt], f32, tag="mv")

            # iters > 1 repeats the whole compute body for loop-differencing
            # timing; the same instructions run N times over the same data.
            loop_ctx = (
                tc.For_i(0, iters, 1) if iters > 1 else contextlib.nullcontext()
            )
            with loop_ctx:
                _body(nc, bass, mybir, variant, xpool, spool, smallp, qpool,
                      psum, xdt, f32, u32, eh, el if variant == "fp16x2" else None,
                      nn, mv, xh_e, xl_e if variant == "fp16x2" else None,
                      emb_e, q_e, phases)
            nc.sync.dma_start(mv_e[:], mv[:])

    nc.compile()
    return nc


def _body(nc, bass, mybir, variant, xpool, spool, smallp, qpool, psum, xdt,
          f32, u32, eh, el, nn, mv, xh_e, xl_e, emb_e, q_e, phases=PHASES):
    if True:
            for ph in range(phases):
                lo, hi = ph * PH_ROWS, (ph + 1) * PH_ROWS
                xh_t = xpool.tile([128, DCH, PH_ROWS], xdt, tag="xh")
                nc.sync.dma_start(
                    xh_t[:], xh_e[:, lo:hi].rearrange("(d p) n -> p d n", p=128)
                )
                if variant == "fp16x2":
                    xl_t = xpool.tile([128, DCH, PH_ROWS], xdt, tag="xl")
                    nc.sync.dma_start(
                        xl_t[:], xl_e[:, lo:hi].rearrange("(d p) n -> p d n", p=128)
                    )
                    passes = [(xh_t, eh), (xl_t, eh), (xh_t, el)]
                else:
                    passes = [(xh_t, eh)]

                for r in range(RT_PH):
                    rg = ph * RT_PH + r
                    sc = spool.tile([128, K], f32, tag="sc")
                    for c in range(CCH):
                        ps = psum.tile([128, NTILE], f32, tag="ps")
                        n_mm = len(passes) * DCH
                        i_mm = 0
                        for stat, mov in passes:
                            for d in range(DCH):
                                nc.tensor.matmul(
                                    ps[:],
                                    stat[:, d, r * 128 : (r + 1) * 128],
                                    mov[:, d, c * NTILE : (c + 1) * NTILE],
                                    start=(i_mm == 0),
                                    stop=(i_mm == n_mm - 1),
                                )
                                i_mm += 1
                        nc.vector.tensor_tensor(
                            out=sc[:, c * NTILE : (c + 1) * NTILE],
                            in0=ps[:],
                            in1=nn[:, c * NTILE : (c + 1) * NTILE],
                            op=mybir.AluOpType.add,
                        )
                    m8 = smallp.tile([128, 8], f32, tag="m8")
                    i8 = smallp.tile([128, 8], u32, tag="i8")
                    nc.vector.max(m8[:], sc[:])
                    nc.vector.max_index(i8[:], m8[:], sc[:])
                    nc.vector.tensor_copy(mv[:, rg : rg + 1], m8[:, 0:1])
                    qt = qpool.tile([128, D], f32, tag="qt")
                    nc.gpsimd.indirect_dma_start(
                        out=qt[:],
                        out_offset=None,
                        in_=emb_e[:],
                        in_offset=bass.IndirectOffsetOnAxis(ap=i8[:, 0:1], axis=0),
                    )
                    nc.scalar.dma_start(q_e[rg * 128 : (rg + 1) * 128, :], qt[:])


def _split16(a):
    ah = a.astype(np.float16)
    al = (a - ah.astype(np.float32)).astype(np.float16)
    return ah, al


def kernel(x, embeddings, trace=False):
    from concourse.bass_utils import run_bass_kernel_spmd

    x = np.ascontiguousarray(np.asarray(x, dtype=np.float32))
    e = np.ascontiguousarray(np.asarray(embeddings, dtype=np.float32))

    variant = VARIANT
    if variant not in _CACHE:
        _CACHE[variant] = _build(variant)
    nc = _CACHE[variant]

    es = e * SCALE
    negnorm = -np.sum(es.astype(np.float64) ** 2, axis=1)
    negnorm_b = np.ascontiguousarray(
        np.broadcast_to(negnorm.astype(np.float32), (128, K))
    )
    if variant == "fp16x2":
        eh, el = _split16(es)
        ehT = np.ascontiguousarray(eh.T)
        elT = np.ascontiguousarray(el.T)
    else:
        ehT = np.ascontiguousarray(es.T)

    in_maps = []
    for i in range(NCORES):
        # 2*SCALE: the cross term needs the factor 2 from -2*x.e; folding it
        # into the x-side prescale makes the device compute
        # S^2 * (2*x.e - ||e||^2) with no extra instructions.
        xs = x[i * ROWS : (i + 1) * ROWS] * (2.0 * SCALE)
        m = {"negnorm": negnorm_b, "emb": e, "eh": ehT}
        if variant == "fp16x2":
            xh, xl = _split16(xs)
            m["xh"] = np.ascontiguousarray(xh.T)
            m["xl"] = np.ascontiguousarray(xl.T)
            m["el"] = elT
        else:
            m["xh"] = np.ascontiguousarray(xs.T)
        in_maps.append(m)

    res = run_bass_kernel_spmd(nc, in_maps, list(range(NCORES)), trace=trace)

    q = np.concatenate([res.results[i]["q"] for i in range(NCORES)], axis=0)

    # loss = 1.25 * mean((q - x)^2); sum_i ||q_i - x_i||^2 = ||x||_F^2 + sum_i min_score_i
    # where min_score_i = min_k(||e_k||^2 - 2 x_i . e_k) = -mv / SCALE^2.
    sum_x2 = 0.0
    for i in range(0, N, 8192):
        xb = x[i : i + 8192].astype(np.float64)
        sum_x2 += float(np.einsum("ij,ij->", xb, xb))
    sum_min = 0.0
    for i in range(NCORES):
        sum_min += -np.sum(res.results[i]["mv"].astype(np.float64)) / float(
            SCALE
        ) ** 2
    loss = np.float32((1.0 + COMMITMENT_COST) * (sum_x2 + sum_min) / (N * D))

    if trace:
        kernel.last_exec_time_ns = res.exec_time_ns
        kernel.last_results = res
    return (q, loss)
